# revision 11
# baseline (speedup 1.0000x reference)
"""DeepViT (6-layer re-attention ViT) Bass/Tile kernel for 8 Trainium2 NeuronCores.

Data-parallel over batch: 32 images -> 4 per core. Full model per core.
Residual stream kept transposed: XT [128(d%128), 6(d//128), 788(img*197+tok)] fp32r.
Attention re-attention mix done on PE via interleaved (j,h) transpose + block-diag
matmul; per-(i,j) LayerNorm-over-heads stats from the mixed tensor; attn@v per head.
"""
import sys, os
sys.path.insert(0, "/opt/trn_rl_repo")
import numpy as np

DEPTH, HEADS, DH = 6, 12, 64
D, MLP, FEAT = 768, 3072, 1000
B, NPATCH = 32, 196
SEQ = NPATCH + 1       # 197 tokens
SEQP = 200             # j padded to 25 chunks of 8
HP = 16                # head slots (12 real + 4 pad)
NCORES = 8
EPS = 1e-5

_CACHE = {}


def _build(L, NIMG):
    import concourse.tile as tile
    import concourse.mybir as mybir
    import concourse.bass as bass
    from concourse import bacc
    from concourse.masks import make_identity

    f32, f32r, bf16 = mybir.dt.float32, mybir.dt.float32r, mybir.dt.bfloat16
    AF = mybir.ActivationFunctionType
    OP = mybir.AluOpType
    X_AX = mybir.AxisListType.X

    TOK = NIMG * SEQ          # 788 for NIMG=4
    NH = [(0, TOK // 2), (TOK // 2, TOK - TOK // 2)]  # token free-dim halves
    ITS = [(0, 128), (128, SEQ - 128)]                # i tiles within one image
    JTS = [(0, 128), (128, SEQ - 128)]                # j tiles

    nc = bacc.Bacc("TRN2", target_bir_lowering=False, debug=False,
                   enable_asserts=True, num_devices=NCORES)

    # ---- DRAM I/O (names match reference.setup_inputs) ----
    dram = {}
    def din(name, shape):
        dram[name] = nc.dram_tensor(name, list(shape), f32, kind="ExternalInput").ap()
        return dram[name]

    din("x", (NIMG, NPATCH, D))
    din("cls_token", (1, 1, D))
    for nm, shp in [("ln1_g", (DEPTH, D)), ("ln1_b", (DEPTH, D)),
                    ("w_qkv", (DEPTH, D, 3 * D)), ("reattn_w", (DEPTH, HEADS, HEADS)),
                    ("reattn_g", (DEPTH, HEADS)), ("reattn_b", (DEPTH, HEADS)),
                    ("w_out", (DEPTH, D, D)), ("b_out", (DEPTH, D)),
                    ("ln2_g", (DEPTH, D)), ("ln2_b", (DEPTH, D)),
                    ("w1", (DEPTH, D, MLP)), ("b1", (DEPTH, MLP)),
                    ("w2", (DEPTH, MLP, D)), ("b2", (DEPTH, D)),
                    ("lnf_g", (D,)), ("lnf_b", (D,)),
                    ("w_head", (D, FEAT)), ("b_head", (FEAT,))]:
        din(nm, shp)
    OUT = nc.dram_tensor("out", [NIMG, FEAT], f32, kind="ExternalOutput").ap()

    with tile.TileContext(nc) as tc:
        import contextlib
        ctx = contextlib.ExitStack()
        with ctx:
            const = ctx.enter_context(tc.tile_pool(name="const", bufs=1))
            persist = ctx.enter_context(tc.tile_pool(name="persist", bufs=1))
            gb = ctx.enter_context(tc.tile_pool(name="gb", bufs=2))
            wq_p = ctx.enter_context(tc.tile_pool(name="wq", bufs=2))
            w2_p = ctx.enter_context(tc.tile_pool(name="w2", bufs=2))
            act = ctx.enter_context(tc.tile_pool(name="act", bufs=1))
            attp = ctx.enter_context(tc.tile_pool(name="attp", bufs=2))
            tsp_p = ctx.enter_context(tc.tile_pool(name="tsp", bufs=3))
            small = ctx.enter_context(tc.tile_pool(name="small", bufs=2))
            ps = ctx.enter_context(tc.tile_pool(name="ps", bufs=1, space="PSUM"))

            def psum(shape, dt, tag, bufs=2):
                return ps.tile(shape, dt, tag=tag, bufs=bufs, name=tag)

            # ---- constants ----
            ident_bf = const.tile([128, 128], bf16)
            make_identity(nc, ident_bf)
            ident_f = const.tile([128, 128], f32)
            make_identity(nc, ident_f)
            ones_f = const.tile([128, 1], f32)
            nc.vector.memset(ones_f, 1.0)
            eps_t = const.tile([128, 1], f32)
            nc.vector.memset(eps_t, EPS)
            ones_r = const.tile([128, 128], f32r)   # all-ones matrix (LN stat replicate)
            one_col = const.tile([128, 1], bf16)    # ones col for colsum-v
            nc.vector.memset(one_col, 1.0)
            onesf_mat = const.tile([128, 128], f32)
            nc.vector.memset(onesf_mat, 1.0)
            nc.vector.tensor_copy(ones_r[:], onesf_mat[:])
            ones_bf = const.tile([128, 128], bf16)
            nc.vector.memset(ones_bf, 1.0)

            # ---- persistent activations ----
            XT = persist.tile([128, 6, TOK], f32r)       # residual (transposed)
            QKVT = persist.tile([128, 18, TOK], bf16)    # qkv (transposed)
            FT = persist.tile([128, 6, TOK // 2], f32)   # ffn2 partial accum (one nh)

            # ============ INIT: build XT from x + cls ============
            clsT = small.tile([128, 6], f32r)
            nc.sync.dma_start(clsT[:], dram["cls_token"].rearrange("a b (c p) -> p (a b c)", p=128).bitcast(f32r))
            for b in range(NIMG):
                nc.vector.tensor_copy(XT[:, :, b * SEQ:b * SEQ + 1], clsT[:, :, None])
            for b in range(NIMG):
                for t0, tt in [(0, 128), (128, NPATCH - 128)]:
                    x_sb = act.tile([128, D], f32, tag="xinit", bufs=1)
                    nc.sync.dma_start(x_sb[:tt, :], dram["x"][b, t0:t0 + tt, :])
                    for c in range(6):
                        pt = psum([128, 128], f32, "mix")
                        nc.tensor.transpose(pt[:, :tt], x_sb[:tt, c * 128:(c + 1) * 128],
                                            ident_f[:tt, :tt])
                        nc.scalar.activation(XT[:, c, b * SEQ + 1 + t0: b * SEQ + 1 + t0 + tt],
                                             pt[:, :tt], AF.Copy)

            # ============ LayerNorm over d (transposed layout) ============
            def emit_ln(SRC, g_pc, b_pc, DST, tokn, nhalves):
                """DST = LN(SRC) * g + b; SRC/DST [128, 6, tokn] f32r tiles."""
                for (n0, nn) in nhalves:
                    s1 = psum([128, TOK // 2], f32, "big")
                    s2 = psum([128, TOK // 2], f32, "big")
                    sq = act.tile([128, 6, tokn], bf16, tag="lnsq", bufs=1)
                    nc.scalar.activation(sq[:, :, n0:n0 + nn], SRC.bitcast(f32)[:, :, n0:n0 + nn],
                                         AF.Square)
                    for c in range(6):
                        nc.tensor.matmul(s1[:, :nn], ones_r[:, :], SRC[:, c, n0:n0 + nn],
                                         start=(c == 0), stop=(c == 5))
                    for c in range(6):
                        nc.tensor.matmul(s2[:, :nn], ones_bf[:, :], sq[:, c, n0:n0 + nn],
                                         start=(c == 0), stop=(c == 5))
                    mrep = small.tile([128, TOK // 2], f32, tag="mrep")
                    nc.vector.tensor_scalar_mul(mrep[:, :nn], s1[:, :nn], 1.0 / D)
                    m2 = small.tile([128, TOK // 2], f32, tag="m2")
                    nc.vector.tensor_mul(m2[:, :nn], mrep[:, :nn], mrep[:, :nn])
                    var = small.tile([128, TOK // 2], f32, tag="var")
                    nc.vector.scalar_tensor_tensor(var[:, :nn], s2[:, :nn], 1.0 / D,
                                                   m2[:, :nn], OP.mult, OP.subtract)
                    nc.scalar.activation(var[:, :nn], var[:, :nn], AF.Sqrt, bias=eps_t[:, :])
                    rrep = small.tile([128, TOK // 2], f32, tag="rrep")
                    nc.vector.reciprocal(rrep[:, :nn], var[:, :nn])
                    nc.vector.tensor_tensor(DST[:, :, n0:n0 + nn], SRC[:, :, n0:n0 + nn],
                                            mrep[:, None, :nn].to_broadcast((128, 6, nn)),
                                            OP.subtract)
                    nc.vector.tensor_tensor(DST[:, :, n0:n0 + nn], DST[:, :, n0:n0 + nn],
                                            rrep[:, None, :nn].to_broadcast((128, 6, nn)),
                                            OP.mult)
                    for c in range(6):
                        nc.vector.scalar_tensor_tensor(
                            DST[:, c, n0:n0 + nn], DST[:, c, n0:n0 + nn], g_pc[:, c:c + 1],
                            b_pc[:, c:c + 1, None].to_broadcast((128, 1, nn))[:, 0, :],
                            OP.mult, OP.add)

            def load_colvec(src_ap, chunks, tag):
                t = gb.tile([128, chunks], f32, tag=tag)
                nc.sync.dma_start(t[:], src_ap.rearrange("(c p) -> p c", p=128))
                return t

            # ============ layers ============
            for l in range(L):
                g1 = load_colvec(dram["ln1_g"][l], 6, "g1")
                b1c = load_colvec(dram["ln1_b"][l], 6, "b1c")
                H1 = act.tile([128, 6, TOK], f32r, tag="HZ", bufs=1, name="H1")
                emit_ln(XT, g1, b1c, H1, TOK, NH)

                # ---- QKV ----
                wq_ap = dram["w_qkv"][l].rearrange("(kc kp) m -> kp kc m", kp=128)
                for m in range(18):
                    wt = wq_p.tile([128, 6, 128], f32r, tag="wqkv")
                    nc.sync.dma_start(wt[:], wq_ap[:, :, m * 128:(m + 1) * 128].bitcast(f32r))
                    for (n0, nn) in NH:
                        pq = psum([128, TOK // 2], f32, "big")
                        for k in range(6):
                            nc.tensor.matmul(pq[:, :nn], wt[:, k, :], H1[:, k, n0:n0 + nn],
                                             start=(k == 0), stop=(k == 5))
                        nc.scalar.activation(QKVT[:, m, n0:n0 + nn], pq[:, :nn], AF.Copy)

                ZT = act.tile([128, 6, TOK], f32r, tag="HZ", bufs=1, name="ZT")
                # ---- re-attention prep: BD matrix + gamma/beta ----
                wl = small.tile([12, 12], f32, tag="wl")
                nc.sync.dma_start(wl[:], dram["reattn_w"][l])
                wbar = small.tile([12, 1], f32, tag="wbar")
                nc.vector.tensor_reduce(wbar[:], wl[:], axis=X_AX, op=OP.add)
                nc.vector.tensor_scalar_mul(wbar[:], wbar[:], 1.0 / HEADS)
                wpad = small.tile([16, 16], f32, tag="wpad")
                nc.vector.memset(wpad[:], 0.0)
                nc.vector.tensor_copy(wpad[:12, :12], wl[:])
                nc.vector.tensor_copy(wpad[:12, 12:13], wbar[:])
                wpad_bf = small.tile([16, 16], bf16, tag="wpadbf")
                nc.vector.tensor_copy(wpad_bf[:], wpad[:])
                BD = gb.tile([128, 128], bf16, tag="BD")
                nc.vector.memset(BD[:], 0.0)
                for jb in range(8):
                    nc.sync.dma_start(BD[jb * 16:(jb + 1) * 16, jb * 16:(jb + 1) * 16],
                                      wpad_bf[:, :])
                gbc = gb.tile([128, 12], f32, tag="gbc")
                nc.sync.dma_start(gbc[:], bass.AP(tensor=dram["reattn_g"].tensor,
                                                  offset=l * HEADS, ap=[[0, 128], [1, HEADS]]))
                bbc = gb.tile([128, 12], f32, tag="bbc")
                nc.sync.dma_start(bbc[:], bass.AP(tensor=dram["reattn_b"].tensor,
                                                  offset=l * HEADS, ap=[[0, 128], [1, HEADS]]))

                # ---- attention per image ----
                for b in range(NIMG):
                    base = b * SEQ
                    AHT = attp.tile([128, 24, SEQ], bf16, tag="AHT", bufs=1)   # (g, jh) -> [j, i]
                    VG = attp.tile([128, 24, DH], bf16, tag="VG", bufs=1)      # (g, jh) -> [j, d]
                    # v transposes
                    for g in range(HEADS):
                        for jh, (j0, jj) in enumerate(JTS):
                            pv = psum([128, DH], bf16, "tp")
                            po_ = (g % 2) * 64
                            nc.tensor.transpose(
                                pv[:jj, :],
                                QKVT[po_:po_ + 64, 12 + g // 2,
                                     base + j0:base + j0 + jj],
                                ident_bf[po_:po_ + 64, po_:po_ + 64])
                            if g % 2:
                                nc.vector.tensor_copy(VG[:jj, g * 2 + jh, :], pv[:jj, :])
                            else:
                                nc.scalar.activation(VG[:jj, g * 2 + jh, :], pv[:jj, :], AF.Copy)

                    for it, (i0, isz) in enumerate(ITS):
                        S = attp.tile([128, SEQP, HP], bf16, tag="S")
                        nc.vector.memset(S[:, SEQ:, :], 0.0)
                        nc.vector.memset(S[:, :SEQ, 12:], 0.0)
                        den = small.tile([128, HP], f32, tag="den")
                        for h in range(HEADS):
                            pd = psum([128, SEQ], f32, "att")
                            nc.tensor.matmul(
                                pd[:isz, :],
                                QKVT[(h % 2) * 64:(h % 2) * 64 + 64, h // 2,
                                     base + i0:base + i0 + isz],
                                QKVT[(h % 2) * 64:(h % 2) * 64 + 64, 6 + h // 2,
                                     base:base + SEQ],
                                start=True, stop=True)
                            nc.scalar.activation(S[:isz, :SEQ, h], pd[:isz, :], AF.Exp,
                                                 scale=float(DH) ** -0.5,
                                                 accum_out=den[:isz, h:h + 1])
                        inv = small.tile([128, 12], f32, tag="inv")
                        nc.vector.reciprocal(inv[:isz, :], den[:isz, :12])
                        for h in range(HEADS):
                            nc.vector.tensor_scalar_mul(S[:isz, :SEQ, h], S[:isz, :SEQ, h],
                                                        inv[:isz, h:h + 1])
                        # transpose -> mix -> U2g [i, g(13), j(256)]
                        U2g = attp.tile([128, 13, 256], bf16, tag="U2g", bufs=1)
                        Sv = S.rearrange("p j h -> p (j h)").rearrange("p (c f) -> p c f", f=128)
                        for c in range(25):
                            pt = psum([128, 128], bf16, "tp")
                            nc.tensor.transpose(pt[:, :isz], Sv[:isz, c, :], ident_bf[:isz, :isz])
                            tsb = tsp_p.tile([128, 128], bf16, tag="TS")
                            if c % 2:
                                nc.vector.tensor_copy(tsb[:, :isz], pt[:, :isz])
                            else:
                                nc.scalar.activation(tsb[:, :isz], pt[:, :isz], AF.Copy)
                            pu = psum([128, 128], f32, "mix")
                            nc.tensor.matmul(pu[:isz, :], tsb[:, :isz], BD[:, :],
                                             start=True, stop=True)
                            if c % 2:
                                nc.scalar.activation(U2g[:isz, :, 8 * c:8 * c + 8],
                                                     pu.rearrange("p (j g) -> p g j", g=16)[:isz, :13, :],
                                                     AF.Copy)
                            else:
                                nc.vector.tensor_copy(U2g[:isz, :, 8 * c:8 * c + 8],
                                                      pu.rearrange("p (j g) -> p g j", g=16)[:isz, :13, :])
                        # stats over g per (i, j)
                        SQt = attp.tile([128, 12, SEQ], bf16, tag="S", bufs=2)
                        nc.scalar.activation(SQt[:isz], U2g[:isz, :12, :SEQ], AF.Square)
                        msq = small.tile([128, SEQ], f32, tag="msq")
                        nc.vector.tensor_reduce(msq[:isz, :], SQt.rearrange("p g j -> p j g")[:isz],
                                                axis=X_AX, op=OP.add)
                        m2a = small.tile([128, SEQ], f32, tag="m2a")
                        nc.vector.tensor_mul(m2a[:isz, :], U2g[:isz, 12, :SEQ], U2g[:isz, 12, :SEQ])
                        var = small.tile([128, SEQ], f32, tag="vara")
                        nc.vector.scalar_tensor_tensor(var[:isz, :], msq[:isz, :], 1.0 / HEADS,
                                                       m2a[:isz, :], OP.mult, OP.subtract)
                        nc.scalar.activation(var[:isz, :], var[:isz, :], AF.Sqrt, bias=eps_t[:isz, :])
                        rr = small.tile([128, SEQ], f32, tag="rr")
                        nc.vector.reciprocal(rr[:isz, :], var[:isz, :])
                        tt_ = small.tile([128, SEQ], f32, tag="tt")
                        nc.vector.tensor_mul(tt_[:isz, :], U2g[:isz, 12, :SEQ], rr[:isz, :])
                        # a_hat = y*r - t  (in place on U2g g<12)
                        nc.vector.tensor_tensor(U2g[:isz, :12, :SEQ], U2g[:isz, :12, :SEQ],
                                                rr[:isz, None, :].to_broadcast((isz, 12, SEQ)),
                                                OP.mult)
                        nc.vector.tensor_tensor(U2g[:isz, :12, :SEQ], U2g[:isz, :12, :SEQ],
                                                tt_[:isz, None, :].to_broadcast((isz, 12, SEQ)),
                                                OP.subtract)
                        # a_hat transposes -> AHT [j, i] per (g, jh)
                        for g in range(HEADS):
                            for jh in range(2):
                                pa = psum([128, 128], bf16, "tp")
                                nc.tensor.transpose(pa[:, :isz],
                                                    U2g[:isz, g, jh * 128:(jh + 1) * 128],
                                                    ident_bf[:isz, :isz])
                                if g % 2:
                                    nc.vector.tensor_copy(AHT[:, g * 2 + jh, i0:i0 + isz], pa[:, :isz])
                                else:
                                    nc.scalar.activation(AHT[:, g * 2 + jh, i0:i0 + isz], pa[:, :isz], AF.Copy)

                    # attn @ v (+ colsum-v for beta term), z scale/bias -> ZT
                    for g in range(HEADS):
                        pcv = psum([64, SEQ], f32, "att")
                        for jh, (j0, jj) in enumerate(JTS):
                            nc.tensor.matmul(pcv[:, :1], VG[:jj, g * 2 + jh, :],
                                             one_col[:jj, :], start=(jh == 0), stop=(jh == 1))
                        bcv = small.tile([64, 1], f32, tag="bcv")
                        nc.vector.tensor_scalar_mul(bcv[:], pcv[:, :1], bbc[:64, g:g + 1])
                        for it, (i0, isz) in enumerate(ITS):
                            pz = psum([64, SEQ], f32, "att")
                            for jh, (j0, jj) in enumerate(JTS):
                                nc.tensor.matmul(pz[:, :isz], VG[:jj, g * 2 + jh, :],
                                                 AHT[:jj, g * 2 + jh, i0:i0 + isz],
                                                 start=(jh == 0), stop=(jh == 1))
                            nc.vector.scalar_tensor_tensor(
                                ZT[(g % 2) * 64:(g % 2) * 64 + 64, g // 2,
                                   base + i0:base + i0 + isz],
                                pz[:, :isz], gbc[:64, g:g + 1],
                                bcv[:, 0:1].to_broadcast((64, isz)),
                                OP.mult, OP.add)

                # ---- out projection + residual ----
                boT = load_colvec(dram["b_out"][l], 6, "boT")
                wo_ap = dram["w_out"][l].rearrange("(kc kp) m -> kp kc m", kp=128)
                for m in range(6):
                    wo = wq_p.tile([128, 6, 128], f32r, tag="wout")
                    nc.sync.dma_start(wo[:], wo_ap[:, :, m * 128:(m + 1) * 128].bitcast(f32r))
                    for (n0, nn) in NH:
                        po = psum([128, TOK // 2], f32, "big")
                        for k in range(6):
                            nc.tensor.matmul(po[:, :nn], wo[:, k, :], ZT[:, k, n0:n0 + nn],
                                             start=(k == 0), stop=(k == 5))
                        nc.vector.scalar_tensor_tensor(
                            XT[:, m, n0:n0 + nn], po[:, :nn], boT[:, m:m + 1],
                            XT[:, m, n0:n0 + nn], OP.add, OP.add)

                # ---- FFN ----
                g2 = load_colvec(dram["ln2_g"][l], 6, "g1")
                b2c = load_colvec(dram["ln2_b"][l], 6, "b1c")
                H1 = act.tile([128, 6, TOK], f32r, tag="HZ", bufs=1, name="H1b")
                emit_ln(XT, g2, b2c, H1, TOK, NH)
                b1T = load_colvec(dram["b1"][l], 24, "b1T")
                b2T = load_colvec(dram["b2"][l], 6, "b2T")
                w1_ap = dram["w1"][l].rearrange("(kc kp) m -> kp kc m", kp=128)
                w2_ap = dram["w2"][l].rearrange("(kc kp) m -> kp kc m", kp=128)
                for (n0, nn) in NH:
                    for mh in range(2):
                        GT = act.tile([128, 12, TOK // 2], bf16, tag="GT", bufs=1)
                        for m in range(12):
                            mm = mh * 12 + m
                            w1t = wq_p.tile([128, 6, 128], f32r, tag="w1t")
                            nc.sync.dma_start(w1t[:], w1_ap[:, :, mm * 128:(mm + 1) * 128].bitcast(f32r))
                            pf = psum([128, TOK // 2], f32, "big")
                            for k in range(6):
                                nc.tensor.matmul(pf[:, :nn], w1t[:, k, :], H1[:, k, n0:n0 + nn],
                                                 start=(k == 0), stop=(k == 5))
                            nc.scalar.activation(GT[:, m, :nn], pf[:, :nn], AF.Gelu,
                                                 bias=b1T[:, mm:mm + 1])
                        for m2 in range(6):
                            w2t = w2_p.tile([128, 12, 128], bf16, tag="w2t")
                            nc.gpsimd.dma_start(w2t[:], w2_ap[:, mh * 12:(mh + 1) * 12,
                                                            m2 * 128:(m2 + 1) * 128])
                            pf2 = psum([128, TOK // 2], f32, "big")
                            for k in range(12):
                                nc.tensor.matmul(pf2[:, :nn], w2t[:, k, :], GT[:, k, :nn],
                                                 start=(k == 0), stop=(k == 11))
                            if mh == 0:
                                nc.vector.tensor_copy(FT[:, m2, :nn], pf2[:, :nn])
                            else:
                                nc.vector.scalar_tensor_tensor(
                                    XT[:, m2, n0:n0 + nn], pf2[:, :nn], b2T[:, m2:m2 + 1],
                                    XT[:, m2, n0:n0 + nn], OP.add, OP.add)
                                nc.vector.tensor_tensor(
                                    XT[:, m2, n0:n0 + nn], XT[:, m2, n0:n0 + nn],
                                    FT[:, m2, :nn], OP.add)

            # ============ head: cls pool + lnf + classifier ============
            P4 = small.tile([128, 6, NIMG], f32r, tag="P4")
            for b in range(NIMG):
                nc.vector.tensor_copy(P4[:, :, b:b + 1], XT[:, :, b * SEQ:b * SEQ + 1])
            gf = load_colvec(dram["lnf_g"], 6, "g1")
            bf = load_colvec(dram["lnf_b"], 6, "b1c")
            HF = small.tile([128, 6, NIMG], f32r, tag="HF")
            emit_ln(P4, gf, bf, HF, NIMG, [(0, NIMG)])
            bh4 = small.tile([NIMG, FEAT], f32, tag="bh4", bufs=1)
            nc.sync.dma_start(bh4[:], bass.AP(tensor=dram["b_head"].tensor, offset=0,
                                              ap=[[0, NIMG], [1, FEAT]]))
            osb = small.tile([NIMG, FEAT], f32, tag="osb", bufs=1)
            wh_ap = dram["w_head"].rearrange("(kc kp) m -> kp kc m", kp=128)
            for nh2 in range(2):
                wh = wq_p.tile([128, 6, 500], f32r, tag="wh", bufs=1)
                nc.sync.dma_start(wh[:], wh_ap[:, :, nh2 * 500:(nh2 + 1) * 500].bitcast(f32r))
                ph = psum([NIMG, 500], f32, "big")
                for k in range(6):
                    nc.tensor.matmul(ph[:, :], HF[:, k, :], wh[:, k, :],
                                     start=(k == 0), stop=(k == 5))
                nc.vector.tensor_tensor(osb[:, nh2 * 500:(nh2 + 1) * 500], ph[:, :],
                                        bh4[:, nh2 * 500:(nh2 + 1) * 500], OP.add)
            nc.sync.dma_start(OUT, osb[:])

    nc.compile()
    return nc


def _get_nc(L=DEPTH, NIMG=B // NCORES):
    key = (L, NIMG)
    if key not in _CACHE:
        _CACHE[key] = _build(L, NIMG)
    return _CACHE[key]


def kernel(**inputs):
    from concourse.bass_utils import run_bass_kernel_spmd
    NIMG = B // NCORES
    nc = _get_nc(DEPTH, NIMG)
    weights = {k: np.ascontiguousarray(np.asarray(v, dtype=np.float32))
               for k, v in inputs.items() if k != "x"}
    x_full = np.asarray(inputs["x"], dtype=np.float32)
    in_maps = []
    for c in range(NCORES):
        m = dict(weights)
        m["x"] = np.ascontiguousarray(x_full[c * NIMG:(c + 1) * NIMG])
        in_maps.append(m)
    res = run_bass_kernel_spmd(nc, in_maps, core_ids=list(range(NCORES)))
    return np.concatenate([r["out"] for r in res.results], axis=0)
